# revision 20
# baseline (speedup 1.0000x reference)
"""Bond-centered tensor-moment descriptor kernel for Trainium2 (8 NeuronCores).

Strategy: edges are sharded 8 ways; every core gets the full (relaid-out)
atom-descriptor table and gathers its edge endpoints with indirect DMA.
The Clebsch-Gordan tensor product is computed as: build Z[e,(f,a,b)] =
sh_a(u)*rad_f(r)*y_b,f with per-partition-scalar ops, transpose Z to
feature-partitions with TensorE, then one stationary matmul per f-pair
whose weights fold CG coefficients and tp_weights.
"""
import math
import numpy as np

import concourse.bass as bass
import concourse.tile as tile
from concourse import mybir
from concourse.bass import AP
from concourse.bass_utils import run_bass_kernel_spmd
from concourse.masks import make_identity
from concourse.tile import TileContext, ScopedClock

# ----------------------------------------------------------------------------
# Problem constants (hardcoded per contract)
# ----------------------------------------------------------------------------
CUTOFF = 5.0
MAX_BASIS_DEG = 2
MAX_DEG = 4
N_ATOMS = 20000
N_EDGES = 50000
F = 16
N_CORES = 8

NSH = (MAX_BASIS_DEG + 1) ** 2        # 9 spherical-harmonic components
NB = (MAX_DEG + 1) ** 2               # 25 atom-feature m-slots
BPAD = 26                             # b padded for 4-byte alignment of a*BPAD
NC_OUT = 2 * NB                       # 50 output (parity, c) slots
ABLK = NSH * BPAD                     # 234 Z-columns per f
FPBLK = 512                           # padded Z-columns per f-pair (2*ABLK=468 -> 512)
ZCOLS = 8 * FPBLK                     # 4096
EPC = 6400                            # edges per core (padded from 6250)
EBLK = 128                            # edges per block
NBLK = EPC // EBLK                    # 50 blocks per core

PATHS = [(l1, l2, l3)
         for l1 in range(MAX_BASIS_DEG + 1)
         for l2 in range(MAX_DEG + 1)
         for l3 in range(abs(l1 - l2), min(l1 + l2, MAX_DEG) + 1)]

ZDT = mybir.dt.bfloat16               # Z / zT / W dtype (flip to float32 if precision demands)
ZNP = np.dtype("bfloat16") if False else None  # host cast handled via ml_dtypes below


# ----------------------------------------------------------------------------
# Clebsch-Gordan coefficients (host, numpy only)
# ----------------------------------------------------------------------------
def _fac(n):
    return math.factorial(n)


def _cg(j1, m1, j2, m2, j3, m3):
    if m1 + m2 != m3:
        return 0.0
    if j3 < abs(j1 - j2) or j3 > j1 + j2:
        return 0.0
    pre = math.sqrt((2 * j3 + 1) * _fac(j3 + j1 - j2) * _fac(j3 - j1 + j2)
                    * _fac(j1 + j2 - j3) / _fac(j1 + j2 + j3 + 1))
    pre *= math.sqrt(_fac(j3 + m3) * _fac(j3 - m3) * _fac(j1 - m1) * _fac(j1 + m1)
                     * _fac(j2 - m2) * _fac(j2 + m2))
    s = 0.0
    for k in range(max(0, j2 - j3 - m1, j1 - j3 + m2),
                   min(j1 + j2 - j3, j1 - m1, j2 + m2) + 1):
        s += (-1) ** k / (_fac(k) * _fac(j1 + j2 - j3 - k) * _fac(j1 - m1 - k)
                          * _fac(j2 + m2 - k) * _fac(j3 - j2 + m1 + k)
                          * _fac(j3 - j1 - m2 + k))
    return pre * s


def _umat(l):
    U = np.zeros((2 * l + 1, 2 * l + 1), dtype=np.complex128)
    s2 = 1.0 / np.sqrt(2.0)
    for m in range(-l, l + 1):
        if m > 0:
            U[m + l, m + l] = ((-1) ** m) * s2
            U[m + l, -m + l] = s2
        elif m == 0:
            U[l, l] = 1.0
        else:
            am = -m
            U[m + l, m + l] = 1j * s2
            U[m + l, am + l] = -1j * ((-1) ** am) * s2
    return U


def _real_cg(l1, l2, l3):
    C = np.zeros((2 * l1 + 1, 2 * l2 + 1, 2 * l3 + 1), dtype=np.complex128)
    for m1 in range(-l1, l1 + 1):
        for m2 in range(-l2, l2 + 1):
            m3 = m1 + m2
            if -l3 <= m3 <= l3:
                C[m1 + l1, m2 + l2, m3 + l3] = _cg(l1, m1, l2, m2, l3, m3)
    G = np.einsum('aA,bB,cC,ABC->abc', _umat(l1), _umat(l2),
                  np.conj(_umat(l3)), C)
    G = G.real if (l1 + l2 + l3) % 2 == 0 else G.imag
    return np.ascontiguousarray(G)


def _build_weight_tensor(tp_weights):
    """W[f, a, b, c, ] -> big [ZCOLS, NC_OUT] matrix in the Z-column order
    (f-major, then a, then padded b), entry = CG[a,b,c] * tp_weights[path, f]."""
    G_abc = np.zeros((NSH, NB, NC_OUT), dtype=np.float64)
    for p, (l1, l2, l3) in enumerate(PATHS):
        G = _real_cg(l1, l2, l3)
        par = (l1 + l2 + l3) % 2
        for ai in range(2 * l1 + 1):
            for bi in range(2 * l2 + 1):
                for ci in range(2 * l3 + 1):
                    v = G[ai, bi, ci]
                    if v != 0.0:
                        ga = l1 * l1 + ai
                        gb = l2 * l2 + bi
                        gc = par * NB + l3 * l3 + ci
                        G_abc[ga, gb, gc] = v
    # per-path tp weight lookup per (a,b,c) triple
    path_idx = {}
    for p, (l1, l2, l3) in enumerate(PATHS):
        path_idx[(l1, l2, l3)] = p
    l_of_a = [0, 1, 1, 1, 2, 2, 2, 2, 2]
    l_of_b = [int(np.sqrt(b)) for b in range(NB)]
    l_of_c = [int(np.sqrt(c % NB)) for c in range(NC_OUT)]

    W = np.zeros((F, NSH, BPAD, NC_OUT), dtype=np.float64)
    for ga in range(NSH):
        for gb in range(NB):
            nz = np.nonzero(G_abc[ga, gb])[0]
            if len(nz) == 0:
                continue
            for gc in nz:
                p = path_idx[(l_of_a[ga], l_of_b[gb], l_of_c[gc])]
                for f in range(F):
                    W[f, ga, gb, gc] = G_abc[ga, gb, gc] * float(tp_weights[p, f])
    W = W.reshape(F, ABLK, NC_OUT)
    # assemble per-f-pair stationaries [FPBLK, 2*NC_OUT] with f block-diag M
    out = np.zeros((8, FPBLK, 2 * NC_OUT), dtype=np.float64)
    for fp in range(8):
        for df in range(2):
            out[fp, df * ABLK:(df + 1) * ABLK, df::2] = W[2 * fp + df]
    return out.reshape(8 * FPBLK, 2 * NC_OUT)


# ----------------------------------------------------------------------------
# Device kernel builder
# ----------------------------------------------------------------------------
_NC_CACHE = {}


def _drain_and_barrier_patched(self, tick_clock, wait_clock):
    # this container's walrus supports only one sync-wait per CTRL
    nc = self.nc
    drain_inst = nc.sync.drain()
    wait_clock.add_sem_waits(drain_inst.ins,
                             ScopedClock({None: tick_clock.global_clock}))
    si = drain_inst.ins.sync_info
    waits = list(si.on_wait) if si else []
    if len(waits) > 1:
        drain_inst.ins.sync_info = mybir.SyncInfo(on_wait=[waits[0]],
                                                  on_update=list(si.on_update))
        for w in waits[1:]:
            d2 = nc.sync.drain()
            d2.ins.sync_info = mybir.SyncInfo(on_wait=[w], on_update=[])
    nc.all_engine_barrier()
    assert self.sems is not None
    popped = nc._tile_sem_poison_stack.pop()
    assert popped is self._sem_poison
    nc.clear_and_free_semaphores(list(self.sems.allocated().values()))
    nc.all_engine_barrier()


TileContext._drain_and_barrier = _drain_and_barrier_patched

# each f-pair owns exactly 4 aligned 128-row zT chunks
KBLK = FPBLK


def _kpieces(fp):
    return [(4 * fp + i, 0, 128) for i in range(4)]


def _split_multi_waits(nc):
    """This container's walrus supports one sync-wait per instruction; move
    extra waits onto injected same-engine NoOps placed just before."""
    for f in nc.m.functions:
        for bb in f.blocks:
            newl = []
            changed = False
            for inst in bb.instructions:
                si = inst.sync_info
                waits = list(si.on_wait) if si else []
                if len(waits) > 1:
                    changed = True
                    for k, w in enumerate(waits[:-1]):
                        nop = mybir.InstDrain(name=f"{inst.name}-sw{k}",
                                              ins=[], outs=[])
                        nop.engine = inst.engine
                        nop.sync_info = mybir.SyncInfo(on_wait=[w], on_update=[])
                        newl.append(nop)
                    inst.sync_info = mybir.SyncInfo(on_wait=[waits[-1]],
                                                    on_update=list(si.on_update))
                newl.append(inst)
            if changed:
                bb.instructions = newl


def _build_bass(split_waits=True):
    nc = bass.Bass("TRN2", target_bir_lowering=False, debug=False)
    dt = mybir.dt
    f32 = dt.float32

    a2 = nc.dram_tensor("a2", [N_ATOMS, F * BPAD], f32, kind="ExternalInput").ap()
    idx = nc.dram_tensor("idx", [EPC, 2], dt.int32, kind="ExternalInput").ap()
    disp = nc.dram_tensor("disp", [EPC, 4], f32, kind="ExternalInput").ap()
    wmat = nc.dram_tensor("wmat", [8 * KBLK, 2 * NC_OUT], f32, kind="ExternalInput").ap()
    out = nc.dram_tensor("out", [EPC, 800], f32, kind="ExternalOutput").ap()

    NCHUNK = (ZCOLS + 127) // 128  # 30 zT chunks (last is 32 rows)
    SUPER = 4                      # e-blocks per superblock
    zdt = ZDT

    from contextlib import ExitStack
    with TileContext(nc) as tc, ExitStack() as ctx:
        consts = ctx.enter_context(tc.tile_pool(name="consts", bufs=1))
        wpool = ctx.enter_context(tc.tile_pool(name="wpool", bufs=1))
        epool = ctx.enter_context(tc.tile_pool(name="epool", bufs=3))   # per-eblock working tiles
        spool = ctx.enter_context(tc.tile_pool(name="spool", bufs=3))   # small per-eblock stats
        zpool = ctx.enter_context(tc.tile_pool(name="zpool", bufs=2))  # x4 tags = 8 slots   # Z tiles
        ztp = ctx.enter_context(tc.tile_pool(name="ztp", bufs=2))       # zT sbuf chunks
        opool = ctx.enter_context(tc.tile_pool(name="opool", bufs=3))   # out sbuf
        osp = ctx.enter_context(tc.tile_pool(name="osp", bufs=2))       # [100,512] staging
        pst = ctx.enter_context(tc.tile_pool(name="pst", bufs=3, space="PSUM"))  # transposes of Z
        psm = ctx.enter_context(tc.tile_pool(name="psm", bufs=2, space="PSUM"))  # z-matmul out
        pso = ctx.enter_context(tc.tile_pool(name="pso", bufs=2, space="PSUM"))  # out transposes

        # ---- constants ----
        ident = consts.tile([128, 128], f32)
        make_identity(nc, ident[:])
        identb = consts.tile([128, 128], zdt)
        make_identity(nc, identb[:])
        biasC = consts.tile([128, 1], f32)
        nc.vector.memset(biasC[:], CUTOFF)
        krow = consts.tile([128, F], f32)
        kint = consts.tile([128, F], dt.int32)
        nc.gpsimd.iota(kint[:], pattern=[[1, F]], base=1, channel_multiplier=0)
        nc.vector.tensor_copy(out=krow[:], in_=kint[:])  # 1..16 as float

        # stationary W tiles, one per (f-pair, piece)
        wt = {}
        for fp in range(8):
            for pi, (chunk, r0, r1) in enumerate(_kpieces(fp)):
                t = wpool.tile([r1 - r0, 2 * NC_OUT], zdt, tag=f"w_{fp}_{pi}",
                               name=f"w_{fp}_{pi}")
                base = fp * KBLK + sum(
                    p[2] - p[1] for p in _kpieces(fp)[:pi])
                nc.gpsimd.dma_start(out=t[:], in_=wmat[base:base + (r1 - r0), :])
                wt[(fp, pi)] = t

        for sb in range((NBLK + SUPER - 1) // SUPER):
            eblocks = [eb for eb in range(sb * SUPER, min((sb + 1) * SUPER, NBLK))]
            zs = []
            for eb in eblocks:
                e0 = eb * EBLK
                # ---- gather both endpoints, summed in-DMA ----
                idx_t = spool.tile([128, 2], dt.int32, tag="idx")
                nc.sync.dma_start(out=idx_t[:], in_=idx[e0:e0 + 128, :])
                y = epool.tile([128, F * BPAD], f32, tag="y")
                nc.gpsimd.indirect_dma_start(
                    out=y[:], out_offset=None, in_=a2[:],
                    in_offset=bass.IndirectOffsetOnAxis(ap=idx_t[:, 0:1], axis=0))
                nc.gpsimd.indirect_dma_start(
                    out=y[:], out_offset=None, in_=a2[:],
                    in_offset=bass.IndirectOffsetOnAxis(ap=idx_t[:, 1:2], axis=0),
                    compute_op=mybir.AluOpType.add)

                # ---- per-edge geometry ----
                d = spool.tile([128, 4], f32, tag="d")
                nc.sync.dma_start(out=d[:], in_=disp[e0:e0 + 128, :])
                sq = spool.tile([128, 3], f32, tag="sq")
                nc.scalar.square(sq[:], d[:, 0:3])
                r2 = spool.tile([128, 1], f32, tag="r2")
                nc.vector.tensor_reduce(out=r2[:], in_=sq[:], op=mybir.AluOpType.add,
                                        axis=mybir.AxisListType.X)
                r = spool.tile([128, 1], f32, tag="r")
                nc.scalar.sqrt(r[:], r2[:])
                rm = spool.tile([128, 1], f32, tag="rm")
                nc.vector.tensor_scalar(out=rm[:], in0=r[:], scalar1=1e-9, scalar2=None,
                                        op0=mybir.AluOpType.max)
                rinv = spool.tile([128, 1], f32, tag="rinv")
                nc.vector.reciprocal(rinv[:], rm[:])
                u = spool.tile([128, 3], f32, tag="u")
                nc.vector.tensor_scalar(out=u[:], in0=d[:, 0:3], scalar1=rinv[:, 0:1], scalar2=None,
                                        op0=mybir.AluOpType.mult)
                # mask = (r < CUTOFF) via sign(C - r): {-1,0,1} -> {0,0.5,1}
                msgn = spool.tile([128, 1], f32, tag="msgn")
                nc.scalar.activation(msgn[:], r[:], mybir.ActivationFunctionType.Sign,
                                     bias=biasC[:, 0:1], scale=-1.0)
                mask = spool.tile([128, 1], f32, tag="mask")
                nc.vector.tensor_scalar(out=mask[:], in0=msgn[:], scalar1=0.5,
                                        scalar2=0.5, op0=mybir.AluOpType.mult,
                                        op1=mybir.AluOpType.add)

                # sh [128, 9]
                c1 = 0.4886025119029199
                c2 = 1.0925484305920792
                sh = spool.tile([128, NSH], f32, tag="sh")
                nc.vector.memset(sh[:, 0:1], 0.28209479177387814)
                nc.vector.tensor_scalar(out=sh[:, 1:2], in0=u[:, 1:2], scalar1=c1, scalar2=None,
                                        op0=mybir.AluOpType.mult)
                nc.vector.tensor_scalar(out=sh[:, 2:3], in0=u[:, 2:3], scalar1=c1, scalar2=None,
                                        op0=mybir.AluOpType.mult)
                nc.vector.tensor_scalar(out=sh[:, 3:4], in0=u[:, 0:1], scalar1=c1, scalar2=None,
                                        op0=mybir.AluOpType.mult)
                # xy, yz, xz
                nc.vector.scalar_tensor_tensor(
                    out=sh[:, 4:5], in0=u[:, 0:1], scalar=c2,
                    in1=u[:, 1:2], op0=mybir.AluOpType.mult, op1=mybir.AluOpType.mult)
                nc.vector.scalar_tensor_tensor(
                    out=sh[:, 5:6], in0=u[:, 1:2], scalar=c2,
                    in1=u[:, 2:3], op0=mybir.AluOpType.mult, op1=mybir.AluOpType.mult)
                nc.vector.scalar_tensor_tensor(
                    out=sh[:, 7:8], in0=u[:, 0:1], scalar=c2,
                    in1=u[:, 2:3], op0=mybir.AluOpType.mult, op1=mybir.AluOpType.mult)
                # 0.3154*(3z^2-1)
                t6 = spool.tile([128, 1], f32, tag="t6")
                nc.vector.scalar_tensor_tensor(
                    out=t6[:], in0=u[:, 2:3], scalar=3.0, in1=u[:, 2:3],
                    op0=mybir.AluOpType.mult, op1=mybir.AluOpType.mult)
                nc.scalar.activation(sh[:, 6:7], t6[:], mybir.ActivationFunctionType.Copy,
                                     bias=-0.31539156525252005, scale=0.31539156525252005)
                # 0.5*c2*(x^2-y^2)
                t8 = spool.tile([128, 1], f32, tag="t8")
                nc.vector.scalar_tensor_tensor(
                    out=t8[:], in0=u[:, 0:1], scalar=0.5 * c2, in1=u[:, 0:1],
                    op0=mybir.AluOpType.mult, op1=mybir.AluOpType.mult)
                t8b = spool.tile([128, 1], f32, tag="t8b")
                nc.vector.scalar_tensor_tensor(
                    out=t8b[:], in0=u[:, 1:2], scalar=-0.5 * c2, in1=u[:, 1:2],
                    op0=mybir.AluOpType.mult, op1=mybir.AluOpType.mult)
                nc.vector.tensor_add(out=sh[:, 8:9], in0=t8[:], in1=t8b[:])

                # rad [128, 16]: sinc(k*r/C) * (r<C)
                x = spool.tile([128, F], f32, tag="x")
                rc = spool.tile([128, 1], f32, tag="rc")
                nc.vector.tensor_scalar(out=rc[:], in0=rm[:], scalar1=1.0 / CUTOFF, scalar2=None,
                                        op0=mybir.AluOpType.mult)
                nc.vector.tensor_scalar(out=x[:], in0=krow[:], scalar1=rc[:, 0:1], scalar2=None,
                                        op0=mybir.AluOpType.mult)
                # sin(pi*t) via range reduction: s = t - 2*int(t/2) (trunc or
                # round both keep sin(pi*s) == sin(pi*t) up to period), s in [-1,1]
                px = spool.tile([128, F], f32, tag="px")
                nc.scalar.activation(px[:], x[:], mybir.ActivationFunctionType.Copy,
                                     bias=0.0, scale=math.pi)
                prec = spool.tile([128, F], f32, tag="prec")
                nc.vector.reciprocal(prec[:], px[:])
                # n = round_nearest(x/2) via the 2^23 magic-number trick,
                # s = x - 2n in [-1, 1]; sin(pi*s) == sin(pi*x) by periodicity
                MAGIC = 8388608.0
                th = spool.tile([128, F], f32, tag="th")
                nc.vector.tensor_scalar(out=th[:], in0=x[:], scalar1=0.5,
                                        scalar2=MAGIC, op0=mybir.AluOpType.mult,
                                        op1=mybir.AluOpType.add)
                tf = spool.tile([128, F], f32, tag="tf")
                nc.vector.tensor_scalar(out=tf[:], in0=th[:], scalar1=-MAGIC,
                                        scalar2=None, op0=mybir.AluOpType.add)
                q = spool.tile([128, F], f32, tag="q")
                nc.vector.scalar_tensor_tensor(
                    out=q[:], in0=tf[:], scalar=-2.0, in1=x[:],
                    op0=mybir.AluOpType.mult, op1=mybir.AluOpType.add)
                sins = spool.tile([128, F], f32, tag="sins")
                nc.scalar.activation(sins[:], q[:], mybir.ActivationFunctionType.Sin,
                                     bias=0.0, scale=math.pi)
                rad = spool.tile([128, F], f32, tag="rad")
                nc.vector.scalar_tensor_tensor(
                    out=rad[:], in0=sins[:], scalar=mask[:, 0:1], in1=prec[:],
                    op0=mybir.AluOpType.mult, op1=mybir.AluOpType.mult)

                # ---- y' = y * rad (broadcast over b) ----
                yp = epool.tile([128, F * BPAD], f32, tag="yp")
                nc.vector.tensor_tensor(
                    out=yp[:].rearrange("p (f b) -> p f b", f=F),
                    in0=y[:].rearrange("p (f b) -> p f b", f=F),
                    in1=rad[:, :, None].to_broadcast([128, F, BPAD]),
                    op=mybir.AluOpType.mult)

                # ---- Z[e, (f, a, b)] = sh_a * y' ----
                ebi = eb - eblocks[0]
                z = zpool.tile([128, ZCOLS], zdt, tag=f"z{ebi}", name=f"z{ebi}")
                zs.append(z)
                zap = z[:]
                ypap = yp[:]
                nc.vector.memset(
                    AP(zap.tensor, zap.offset + 2 * ABLK,
                       [list(zap.ap[0]), [FPBLK, 8], [1, FPBLK - 2 * ABLK]]), 0.0)
                for a in range(NSH):
                    zsl = AP(zap.tensor, zap.offset + a * BPAD,
                             [list(zap.ap[0]), [FPBLK, 8], [ABLK, 2], [1, BPAD]])
                    ysl = AP(ypap.tensor, ypap.offset,
                             [list(ypap.ap[0]), [2 * BPAD, 8], [BPAD, 2], [1, BPAD]])
                    nc.vector.tensor_scalar(
                        out=zsl, in0=ysl,
                        scalar1=sh[:, a:a + 1], scalar2=None, op0=mybir.AluOpType.mult)

                # ---- transpose Z into zT chunks ----
            nebs = len(eblocks)
            ne = nebs * 128
            # ---- transpose all Z chunks (one wide psum->sbuf copy per chunk) ----
            zts = [ztp.tile([128, 512], zdt, tag=f"zt_{c}", name=f"zt_{c}")
                   for c in range(NCHUNK)]
            for c in range(NCHUNK):
                pt = pst.tile([128, 512], zdt, tag="pt", space="PSUM")
                for ebi in range(nebs):
                    nc.tensor.transpose(out=pt[:, ebi * 128:(ebi + 1) * 128],
                                        in_=zs[ebi][:, c * 128:(c + 1) * 128],
                                        identity=identb[:])
                nc.vector.tensor_copy(out=zts[c][:, :ne], in_=pt[:, :ne])
            # ---- z-matmul per f-pair + transpose back ----
            outs = [opool.tile([128, 800], f32, tag=f"os_{i}", name=f"os_{i}")
                    for i in range(nebs)]
            for fp in range(8):
                po = psm.tile([NC_OUT * 2, 512], f32, tag="po", space="PSUM")
                pieces = _kpieces(fp)
                for pi, (chunk, r0, r1) in enumerate(pieces):
                    nc.tensor.matmul(
                        out=po[:, :ne], lhsT=wt[(fp, pi)][:],
                        rhs=zts[chunk][r0:r1, :ne],
                        start=(pi == 0), stop=(pi == len(pieces) - 1))
                og = osp.tile([NC_OUT * 2, 512], f32, tag="og")
                nc.scalar.copy(out=og[:, :ne], in_=po[:, :ne])
                for ebi in range(nebs):
                    pt2 = pso.tile([128, NC_OUT * 2], f32, tag="pt2", space="PSUM")
                    nc.tensor.transpose(out=pt2[:, :],
                                        in_=og[:, ebi * 128:(ebi + 1) * 128],
                                        identity=ident[:NC_OUT * 2, :NC_OUT * 2])
                    # scatter into out sbuf: col = c*16 + 2*fp + df
                    orr = outs[ebi][:].rearrange("p (c k) -> p c k", k=16)
                    nc.vector.tensor_copy(
                        out=orr[:, :, 2 * fp:2 * fp + 2],
                        in_=pt2[:].rearrange("p (c t) -> p c t", t=2))
            for ebi, eb in enumerate(eblocks):
                e0 = eb * EBLK
                nc.sync.dma_start(out=out[e0:e0 + 128, :], in_=outs[ebi][:])

    if split_waits:
        _split_multi_waits(nc)
    return nc


def _get_nc():
    if "nc" not in _NC_CACHE:
        _NC_CACHE["nc"] = _build_bass()
    return _NC_CACHE["nc"]


# ----------------------------------------------------------------------------
# Host entry point
# ----------------------------------------------------------------------------
def kernel(atomic_descriptors, tp_weights, neighbour_displacements,
           neighbour_indices):
    atomic_descriptors = np.asarray(atomic_descriptors, dtype=np.float32)
    tp_weights = np.asarray(tp_weights, dtype=np.float32)
    neighbour_displacements = np.asarray(neighbour_displacements, dtype=np.float32)
    neighbour_indices = np.asarray(neighbour_indices, dtype=np.int32)

    # relayout atom table: (A, 1, 25, 16) -> (A, 16, 26) f-major, b padded
    A = atomic_descriptors.reshape(N_ATOMS, NB, F)
    a2 = np.zeros((N_ATOMS, F, BPAD), dtype=np.float32)
    a2[:, :, :NB] = A.transpose(0, 2, 1)
    a2 = a2.reshape(N_ATOMS, F * BPAD)

    wmat = _build_weight_tensor(tp_weights).astype(np.float32)

    in_maps = []
    shard = N_EDGES // N_CORES
    for c in range(N_CORES):
        idx = np.zeros((EPC, 2), dtype=np.int32)
        disp = np.zeros((EPC, 4), dtype=np.float32)
        idx[:shard] = neighbour_indices[c * shard:(c + 1) * shard]
        d = neighbour_displacements[c * shard:(c + 1) * shard]
        disp[:shard, :3] = d
        disp[shard:, :3] = 1.0  # harmless dummy
        in_maps.append({"a2": a2, "idx": idx, "disp": disp, "wmat": wmat})

    global _last_in_maps
    _last_in_maps = in_maps
    nc = _get_nc()
    res = run_bass_kernel_spmd(nc, in_maps, core_ids=list(range(N_CORES)))

    out = np.empty((N_EDGES, 2, NB, F), dtype=np.float32)
    for c in range(N_CORES):
        o = res.results[c]["out"][:shard].reshape(shard, 2, NB, F)
        out[c * shard:(c + 1) * shard] = o
    return out


if __name__ == "__main__":
    rng = np.random.default_rng(0)
    inputs = {
        "atomic_descriptors": rng.standard_normal((N_ATOMS, 1, NB, F), dtype=np.float32),
        "tp_weights": (rng.standard_normal((len(PATHS), F)) * 0.1).astype(np.float32),
        "neighbour_displacements": (rng.standard_normal((N_EDGES, 3)) * 1.5).astype(np.float32),
        "neighbour_indices": rng.integers(0, N_ATOMS, (N_EDGES, 2)).astype(np.int32),
    }
    out = kernel(**inputs)
    print("kernel ran, out shape", out.shape)
